# revision 1
# baseline (speedup 1.0000x reference)
"""Builds the Bass/Tile kernel for the sharded NT-Xent contrastive loss.

Per-core computation (core c of 8), B=8192 D=512 M=1024:
  inputs (host pre-transposed):
    vt [512, 1024] f32 : v[c*M:(c+1)*M, :].T   (this core's v-shard, transposed)
    ut [512, 8192] f32 : u.T                    (full u, transposed)
    us [512, 1024] f32 : u[c*M:(c+1)*M, :].T   (u-shard cols, for the precise diagonal)
  output:
    loss [1024] f32 : loss rows c*M:(c+1)*M

  math:
    inv_u[j] = 1/||u_j||, inv_v[i] = 1/||v_i||, inv_us[i] = 1/||u_{c*M+i}||
    un = u^T * inv_u (bf16)        vb = v^T cast bf16 (NOT normalized)
    S[i, j] = sum_d vb[d,i] * un[d,j]            (PE, bf16, psum f32)
    den[i]  = sum_j exp(2*inv_v[i] * S[i,j])     (ACT exp, per-partition scale, accum_out)
    dot[i]  = sum_d vt[d,i]*us[d,i]  (f32, precise diagonal)
    s2[i]   = 2*inv_v[i]*inv_us[i]
    loss[i] = log(exp(s2*dot) + den) - s2*dot
"""

from contextlib import ExitStack

import concourse.bass as bass
import concourse.tile as tile
from concourse import bacc, mybir

F32 = mybir.dt.float32
BF16 = mybir.dt.bfloat16
MULT = mybir.AluOpType.mult
ADD = mybir.AluOpType.add
SUB = mybir.AluOpType.subtract
AF = mybir.ActivationFunctionType

B = 8192
D = 512
NCORES = 8
M = B // NCORES  # 1024
KT = D // 128    # 4 d-tiles
NIT = M // 128   # 8 i-tiles


def _rsqrt(nc, pool, ss, cols, tag, newton=2):
    """rsqrt(ss) for a [128, cols] f32 tile; ACT sqrt seed + Newton polish."""
    inv = pool.tile([128, cols], F32, tag=f"{tag}_i")
    nc.vector.reciprocal(inv[:], ss[:])
    y = pool.tile([128, cols], F32, tag=f"{tag}_y0")
    nc.scalar.sqrt(y[:], inv[:])
    for k in range(newton):
        t1 = pool.tile([128, cols], F32, tag=f"{tag}_t{k}")
        nc.vector.tensor_tensor(t1[:], y[:], y[:], MULT)             # y^2
        nc.vector.tensor_tensor(t1[:], t1[:], ss[:], MULT)           # s*y^2
        nc.vector.tensor_scalar(t1[:], t1[:], -0.5, 1.5, MULT, ADD)  # 1.5-0.5*s*y^2
        y2 = pool.tile([128, cols], F32, tag=f"{tag}_y{k + 1}")
        nc.vector.tensor_tensor(y2[:], y[:], t1[:], MULT)
        y = y2
    return y


def build_nc(jchunk=2048, uchunk=2048):
    nc = bacc.Bacc("TRN2", target_bir_lowering=False, debug=False, num_devices=NCORES)

    vt = nc.dram_tensor("vt", [D, M], F32, kind="ExternalInput")
    ut = nc.dram_tensor("ut", [D, B], F32, kind="ExternalInput")
    us = nc.dram_tensor("us", [D, M], F32, kind="ExternalInput")
    loss = nc.dram_tensor("loss", [M], F32, kind="ExternalOutput")
    # DRAM bounce buffers for the small [1, M] -> [128, M/128] transposes
    bounce = [nc.dram_tensor(f"bounce{i}", [M], F32) for i in range(3)]

    n_rounds = B // jchunk           # j-rounds in main loop per i-tile
    mm_per_round = jchunk // 512     # matmuls per d-tile per round

    with tile.TileContext(nc) as tc, ExitStack() as ctx:
        # ---- pools that live for the whole kernel
        consts = ctx.enter_context(tc.tile_pool(name="consts", bufs=1))
        upool = ctx.enter_context(tc.tile_pool(name="upool", bufs=1))
        vpool = ctx.enter_context(tc.tile_pool(name="vpool", bufs=1))
        keep = ctx.enter_context(tc.tile_pool(name="keep", bufs=1))

        # all-ones stationary operands: matmul with these as lhsT produces
        # column sums of rhs, replicated across all 128 output partitions
        ones_bf = consts.tile([128, 128], BF16)
        nc.vector.memset(ones_bf[:], 1.0)
        ones_f = consts.tile([128, 128], F32)
        nc.vector.memset(ones_f[:], 1.0)

        # persistent outputs of the prologue
        inv2v = keep.tile([128, NIT], F32)    # 2/||v_i||
        s2 = keep.tile([128, NIT], F32)       # 2/(||v_i||*||u_i||)
        dot_t = keep.tile([128, NIT], F32)    # v_i . u_i  (raw, f32)
        invu_bc = keep.tile([128, B], BF16)   # 1/||u_j||, same on every partition
        ub = [upool.tile([128, B], BF16, tag=f"ub{dt}", name=f"ub{dt}") for dt in range(KT)]
        vb = [vpool.tile([128, M], BF16, tag=f"vb{dt}", name=f"vb{dt}") for dt in range(KT)]

        # =================== PROLOGUE (scoped pools) ===================
        with tc.tile_pool(name="pstage", bufs=2) as stpool, \
             tc.tile_pool(name="psq", bufs=3) as sqpool, \
             tc.tile_pool(name="pflat", bufs=2) as flpool, \
             tc.tile_pool(name="psmall", bufs=1) as small, \
             tc.tile_pool(name="ppsum", bufs=1, space="PSUM") as pps, \
             tc.tile_pool(name="ppsumv", bufs=3, space="PSUM") as ppsv:

            # ---- load u, cast to bf16
            for dt in range(KT):
                for ch in range(B // uchunk):
                    stage = stpool.tile([128, uchunk], F32, tag="ustage")
                    nc.sync.dma_start(
                        stage[:],
                        ut.ap()[dt * 128:(dt + 1) * 128,
                                ch * uchunk:(ch + 1) * uchunk])
                    nc.vector.tensor_copy(
                        ub[dt][:, ch * uchunk:(ch + 1) * uchunk], stage[:])

            # ---- u sum-of-squares -> 1/||u_j|| replicated in invu_bc
            for r in range(B // 512):
                ps = pps.tile([128, 512], F32, tag="ss", bufs=2)
                for dt in range(KT):
                    sq = sqpool.tile([128, 512], BF16, tag="usq")
                    nc.vector.tensor_tensor(
                        sq[:],
                        ub[dt][:, r * 512:(r + 1) * 512],
                        ub[dt][:, r * 512:(r + 1) * 512], MULT)
                    nc.tensor.matmul(
                        ps[:], lhsT=ones_bf[:], rhs=sq[:],
                        start=(dt == 0), stop=(dt == KT - 1))
                rc = sqpool.tile([128, 512], F32, tag="urc")
                nc.vector.reciprocal(rc[:], ps[:])
                nc.scalar.sqrt(invu_bc[:, r * 512:(r + 1) * 512], rc[:])

            # ---- load v/us; squares + dot; cast vb
            ps_v = ppsv.tile([128, M], F32, tag="vv")
            ps_us = ppsv.tile([128, M], F32, tag="vv")
            ps_dot = ppsv.tile([128, M], F32, tag="vv")
            for dt in range(KT):
                vstage = stpool.tile([128, M], F32, tag="vstage")
                nc.sync.dma_start(vstage[:], vt.ap()[dt * 128:(dt + 1) * 128, :])
                usstage = stpool.tile([128, M], F32, tag="usstage")
                nc.sync.dma_start(usstage[:], us.ap()[dt * 128:(dt + 1) * 128, :])
                nc.vector.tensor_copy(vb[dt][:], vstage[:])

                for name, ps_acc, a, b_ in (
                    ("v2", ps_v, vstage, vstage),
                    ("u2", ps_us, usstage, usstage),
                    ("vu", ps_dot, vstage, usstage),
                ):
                    pr = sqpool.tile([128, M], F32, tag="prod", name=f"prod{name}")
                    nc.vector.tensor_tensor(pr[:], a[:], b_[:], MULT)
                    for jc in range(M // 512):
                        nc.tensor.matmul(
                            ps_acc[:, jc * 512:(jc + 1) * 512], lhsT=ones_f[:],
                            rhs=pr[:, jc * 512:(jc + 1) * 512],
                            start=(dt == 0), stop=(dt == KT - 1))

            # ---- bounce the three [1, M] colsum rows through DRAM to get the
            #      per-partition compact layout [128, 8] (i = t*128 + p)
            ssv_t = small.tile([128, NIT], F32, tag="ssv_t")
            ssus_t = small.tile([128, NIT], F32, tag="ssus_t")
            for k, (name, ps_acc, dst) in enumerate(
                    (("v2", ps_v, ssv_t), ("u2", ps_us, ssus_t),
                     ("vu", ps_dot, dot_t))):
                fl = flpool.tile([1, M], F32, tag="flat", name=f"flat{name}")
                nc.scalar.copy(fl[:], ps_acc[0:1, :])
                nc.sync.dma_start(bounce[k].ap(), fl[:])
                nc.sync.dma_start(
                    dst[:], bounce[k].ap().rearrange("(t p) -> p t", p=128))

            # ---- rsqrt (Newton-polished; these feed the numerator path)
            invv_t = _rsqrt(nc, small, ssv_t, NIT, "rv")        # [128, 8]
            invus_t = _rsqrt(nc, small, ssus_t, NIT, "rs")      # [128, 8]

            nc.vector.tensor_scalar(inv2v[:], invv_t[:], 2.0, None, MULT)
            nc.vector.tensor_tensor(s2[:], inv2v[:], invus_t[:], MULT)

            # ---- normalize u in place (invu_bc already replicated per partition)
            for dt in range(KT):
                nc.vector.tensor_tensor(ub[dt][:], ub[dt][:], invu_bc[:], MULT)

        # =================== MAIN LOOP ===================
        with tc.tile_pool(name="mex", bufs=2) as expool, \
             tc.tile_pool(name="mdp", bufs=2 * n_rounds) as dpool, \
             tc.tile_pool(name="mf", bufs=4) as fpool, \
             tc.tile_pool(name="mpsum", bufs=2, space="PSUM") as mps:

            for it in range(NIT):
                dparts = []
                for r in range(n_rounds):
                    ps = mps.tile([128, jchunk], F32, tag="mm")
                    for dt in range(KT):
                        for jc in range(mm_per_round):
                            j0 = r * jchunk + jc * 512
                            nc.tensor.matmul(
                                ps[:, jc * 512:(jc + 1) * 512],
                                lhsT=vb[dt][:, it * 128:(it + 1) * 128],
                                rhs=ub[dt][:, j0:j0 + 512],
                                start=(dt == 0), stop=(dt == KT - 1))
                    ex = expool.tile([128, jchunk], BF16, tag="ex")
                    dp = dpool.tile([128, 1], F32, tag="dp")
                    nc.scalar.activation(ex[:], ps[:], AF.Exp,
                                         scale=inv2v[:, it:it + 1],
                                         accum_out=dp[:])
                    dparts.append(dp)

                den = fpool.tile([128, 1], F32, tag="den")
                nc.vector.tensor_tensor(den[:], dparts[0][:], dparts[1][:], ADD)
                for k in range(2, n_rounds):
                    nc.vector.tensor_tensor(den[:], den[:], dparts[k][:], ADD)

                numt = fpool.tile([128, 1], F32, tag="num")
                nc.scalar.activation(numt[:], dot_t[:, it:it + 1], AF.Exp,
                                     scale=s2[:, it:it + 1])
                dtot = fpool.tile([128, 1], F32, tag="dtot")
                nc.vector.tensor_tensor(dtot[:], den[:], numt[:], ADD)
                lg = fpool.tile([128, 1], F32, tag="lg")
                nc.scalar.activation(lg[:], dtot[:], AF.Ln)
                t2 = fpool.tile([128, 1], F32, tag="t2")
                nc.vector.tensor_scalar(t2[:], dot_t[:, it:it + 1],
                                        s2[:, it:it + 1], None, MULT)
                lt = fpool.tile([128, 1], F32, tag="lt")
                nc.vector.tensor_tensor(lt[:], lg[:], t2[:], SUB)
                nc.sync.dma_start(
                    loss.ap().rearrange("(t p) -> t p", p=128)[it:it + 1, :],
                    lt[:])

    nc.compile()
    return nc


# ======================================================================
# Host-side entry point: full inputs in, full output out.
# Shards rows of v across the 8 cores; every core gets the full u.
# ======================================================================
import numpy as np

_NC_CACHE = {}


def _get_nc():
    if "nc" not in _NC_CACHE:
        _NC_CACHE["nc"] = build_nc()
    return _NC_CACHE["nc"]


def kernel(v: np.ndarray, u: np.ndarray) -> np.ndarray:
    from concourse.bass_utils import run_bass_kernel_spmd

    nc = _get_nc()
    v = np.asarray(v, dtype=np.float32)
    u = np.asarray(u, dtype=np.float32)
    vT = np.ascontiguousarray(v.T)          # [D, B]
    uT = np.ascontiguousarray(u.T)          # [D, B]
    in_maps = []
    for c in range(NCORES):
        sl = slice(c * M, (c + 1) * M)
        in_maps.append({
            "vt": np.ascontiguousarray(vT[:, sl]),
            "ut": uT,
            "us": np.ascontiguousarray(uT[:, sl]),
        })
    res = run_bass_kernel_spmd(nc, in_maps, core_ids=list(range(NCORES)))
    return np.concatenate([res.results[c]["loss"] for c in range(NCORES)])

